# revision 44
# baseline (speedup 1.0000x reference)
"""LongcatMoe Trainium2 kernel — expert-parallel sparse MoE across 8 NeuronCores.

Strategy (expert-parallel, per the sharding hint):
  - Host computes the tiny router (fp64 softmax/top-k, ~34 MFLOP) and
    dispatches tokens by top-k expert id: core e receives the tokens routed
    to expert e (padded to capacity C), plus expert e's weights.
  - Each core runs the silu-gated MLP for its expert on its token block in
    fp8 e3m4 (4-bit mantissa) with fp32 PSUM accumulation:
      y[:, t] = ((silu(Wg.T x_t)) * (Wu.T x_t)).T @ Wd     in [H, C] layout.
    e3m4 matmuls run at full bf16 PE rate (FWL weight loads) but halve HBM
    traffic, which removes the DMA bottleneck of the bf16 version.
  - Scaling: x*2, W*128 on host (exact powers of two); the PSUM descale
    (1/256) folds into the scalar-engine activations and the residual 128
    folds into the host-side combine weights, so no extra device ops.
  - Host combines in fp64: out[tok] += (gate_w/128) * y, plus the
    zero-expert (identity) term zero_w[t] * x[t] computed exactly on host.

All device tensors are partition-major so every DMA moves >=2KB-ish
contiguous runs per partition, and each logical tensor is ONE SBUF tile
loaded by a few chunked DMAs (fewer ring-paced issue instructions, and a
short tile-pool epilogue: the TileContext teardown semaphore chain scales
with the slot count).

  xT  [128, HO, C]        xT[p, h, t]   = e3m4(2 * x[idx[t], h*128+p])
  wg  [128, IO, HO, 128]  wg[p, j, h, c] = e3m4(128 * w_gate[h*128+p, j*128+c])
  wu  same as wg
  wd  [128, HO, IO, 128]  wd[p, k, j, c] = e3m4(128 * w_down[j*128+p, k*128+c])
  y   [128, HO, C] bf16   y[p, k, t]    = 128 * down[idx[t], k*128+p]
"""

import os

import numpy as np
import ml_dtypes

T, H, I, E, Z, TOPK = 1024, 2048, 1024, 8, 8, 4
ROUTED_SCALING = 1.0
N_CORES = 8
P = 128
HO = H // P  # 16
IO = I // P  # 8
C = 280      # per-expert token capacity on device (seed-0 max is 278)
X_SC = 2.0   # host scale on x before e3m4 quantization
W_SC = 128.0  # host scale on all weights before e3m4 quantization

_PROGRAM = None
LAST_RESULTS = None  # BassKernelResults of the most recent run (for test harness)


def _build_program():
    import concourse.mybir as mybir
    import concourse.tile as tile
    from concourse import bacc

    f32 = mybir.dt.float32
    bf16 = mybir.dt.bfloat16
    fp8 = mybir.dt.float8e3
    fp8e4 = mybir.dt.float8e4
    DR = mybir.MatmulPerfMode.DoubleRow
    SILU = mybir.ActivationFunctionType.Silu
    COPY = mybir.ActivationFunctionType.Copy
    DESC = 1.0 / (X_SC * W_SC)  # PSUM descale for gate/up

    nc = bacc.Bacc(
        "TRN2",
        target_bir_lowering=False,
        debug=False,
        enable_asserts=False,
        num_devices=N_CORES,
    )
    # x rows padded 280->288 so the DoubleRow moving AP's h-pair stride is
    # 16-aligned (DR constraint).
    CP = 288
    xT = nc.dram_tensor("xT", [P, HO, CP], fp8e4, kind="ExternalInput").ap()
    wg = nc.dram_tensor("wg", [P, IO, HO, P], fp8e4, kind="ExternalInput").ap()
    wu = nc.dram_tensor("wu", [P, IO, HO, P], fp8e4, kind="ExternalInput").ap()
    wd = nc.dram_tensor("wd", [P, HO, IO, P], fp8, kind="ExternalInput").ap()
    y = nc.dram_tensor("y", [P, HO, C], bf16, kind="ExternalOutput").ap()

    with tile.TileContext(nc) as tc:
        with (
            tc.tile_pool(name="px", bufs=1) as px,
            tc.tile_pool(name="pwg", bufs=1) as pwg,
            tc.tile_pool(name="pwu", bufs=1) as pwu,
            tc.tile_pool(name="pwd", bufs=1) as pwd,
            tc.tile_pool(name="pmid", bufs=1) as pmid,
            tc.tile_pool(name="psg", bufs=2) as psg,
            tc.tile_pool(name="psu", bufs=2) as psu,
            tc.tile_pool(name="py", bufs=2) as py,
            tc.tile_pool(name="pwrm", bufs=1) as pwrm,
            tc.tile_pool(name="ppg", bufs=3, space="PSUM") as ppg,
            tc.tile_pool(name="ppu", bufs=2, space="PSUM") as ppu,
            tc.tile_pool(name="ppd", bufs=3, space="PSUM") as ppd,
        ):
            # Warmup source: memset on gpsimd (ready earliest in preamble).
            wtile = pwrm.tile([P, C], bf16)
            nc.gpsimd.memset(wtile[:], 0.0)

            xt = px.tile([P, HO, CP], fp8e4)
            wg_t = pwg.tile([P, IO, HO, P], fp8e4)
            wu_t = pwu.tile([P, IO, HO, P], fp8e4)
            wd_t = pwd.tile([P, HO, IO, P], fp8)
            midt = pmid.tile([P, IO, C], fp8)

            # sync (SP) ring: wg0, x chunks, wg1..7, later y out. Fine-grained
            # early chunks so the first gate matmuls start as soon as
            # possible; later weights ride behind at ring rate.
            # The two tensors gating the first matmul group (wg0 and x) ride
            # DIFFERENT rings so their transfers overlap: sync carries all of
            # wg, scalar carries x then wu.
            for j in range(IO):
                nc.sync.dma_start(wg_t[:, j], wg[:, j])
            nc.scalar.dma_start(xt[:, 0:8, :], xT[:, 0:8, :])
            nc.scalar.dma_start(xt[:, 8:, :], xT[:, 8:, :])
            # Tiny dummy activation pulls the scalar ACT_TABLE_LOAD into the
            # preamble instead of stalling the first real silu.
            dumy = pwrm.tile([P, 4], f32, tag="dum")
            nc.scalar.activation(dumy[:], wtile[:, :4], SILU)
            # All wu up front: the scalar ring carries only wu (2.1MB), so
            # the 8 issues clear the queue well before the first activation,
            # and the ring FIFO delivers wu_j at full rate for the faster
            # DoubleRow phase-1 pace.
            for j in range(IO):
                nc.scalar.dma_start(wu_t[:, j], wu[:, j])

            # PE warmup: keep the tensor engine busy while the first input
            # DMAs land so the HAM clock-gate promotes to 8/8 (2.4 GHz)
            # before (not during) the real matmuls.
            pwm = ppd.tile([P, C], f32, tag="pd")
            NWARM = 16
            for w in range(NWARM):
                nc.tensor.matmul(pwm[:], wtile[:, :P], wtile[:],
                                 start=(w == 0), stop=(w == NWARM - 1))

            # Phase 1: mid[j] = silu(x @ Wg_j) * (x @ Wu_j) in [I, C] layout,
            # e4m3 DoubleRow matmuls (each contracts two h-planes).
            # PSUM holds 256*gate and 256*up; the activations descale.
            for j in range(IO):
                pg = ppg.tile([P, C], f32)
                pu = ppu.tile([P, C], f32)
                for hh in range(HO // 2):
                    nc.tensor.matmul(
                        pg[:], wg_t[:, j, 2 * hh:2 * hh + 2, :],
                        xt[:, 2 * hh:2 * hh + 2, :C],
                        start=(hh == 0), stop=(hh == HO // 2 - 1),
                        perf_mode=DR,
                    )
                for hh in range(HO // 2):
                    nc.tensor.matmul(
                        pu[:], wu_t[:, j, 2 * hh:2 * hh + 2, :],
                        xt[:, 2 * hh:2 * hh + 2, :C],
                        start=(hh == 0), stop=(hh == HO // 2 - 1),
                        perf_mode=DR,
                    )
                sg = psg.tile([P, C], f32)
                nc.scalar.activation(sg[:], pg[:], SILU, scale=DESC)
                su = psu.tile([P, C], f32)
                nc.scalar.activation(su[:], pu[:], COPY, scale=DESC)
                # wd rides the sync ring (FIFO behind the wg transfers, so it
                # can never delay phase 1) — the scalar ring stays wu-only,
                # which is what paces the up_j matmuls.
                if 1 <= j <= 4:
                    cnk = j - 1
                    nc.sync.dma_start(wd_t[:, 4 * cnk:4 * cnk + 4],
                                      wd[:, 4 * cnk:4 * cnk + 4])
                nc.vector.tensor_mul(out=midt[:, j, :], in0=sg[:], in1=su[:])

            # Phase 2: y[k] = sum_j Wd[j, k].T @ mid[j] in [H, C] layout
            # (= 128 * true output; host divides the combine weights).
            # y leaves in 2-k chunks so the final transfer after the last
            # matmul is small.
            ty = None
            for k in range(HO):
                if k % 2 == 0:
                    ty = py.tile([P, 2, C], bf16)
                pd = ppd.tile([P, C], f32, tag="pd")
                for j in range(IO):
                    nc.tensor.matmul(
                        pd[:], wd_t[:, k, j, :], midt[:, j, :],
                        start=(j == 0), stop=(j == IO - 1),
                    )
                nc.vector.tensor_copy(out=ty[:, k % 2, :], in_=pd[:])
                if k % 2 == 1:
                    nc.sync.dma_start(y[:, k - 1:k + 1, :], ty[:])

    nc.compile()
    return nc


def _route(x, router_w, corr_bias):
    """fp64 router: returns (topk_idx [T,K], topk_w [T,K])."""
    xl = x.astype(np.float64)
    logits = xl @ router_w.astype(np.float64).T
    logits -= logits.max(axis=1, keepdims=True)
    p = np.exp(logits)
    p /= p.sum(axis=1, keepdims=True)
    sel = p + corr_bias.astype(np.float64)
    topk_idx = np.argsort(-sel, axis=1, kind="stable")[:, :TOPK]
    topk_w = np.take_along_axis(p, topk_idx, axis=1) * ROUTED_SCALING
    return topk_idx, topk_w


def kernel(hidden_states, router_w, corr_bias, w_gate, w_up, w_down):
    global _PROGRAM, LAST_RESULTS
    x = np.asarray(hidden_states, dtype=np.float32)
    router_w = np.asarray(router_w, dtype=np.float32)
    corr_bias = np.asarray(corr_bias, dtype=np.float32)
    w_gate = np.asarray(w_gate, dtype=np.float32)
    w_up = np.asarray(w_up, dtype=np.float32)
    w_down = np.asarray(w_down, dtype=np.float32)

    topk_idx, topk_w = _route(x, router_w, corr_bias)
    routed = topk_idx < E
    zero_w = (topk_w * (~routed)).sum(axis=1)  # [T] fp64

    f8 = ml_dtypes.float8_e3m4
    f8e4 = ml_dtypes.float8_e4m3
    x8 = (x.astype(np.float64) * X_SC).astype(f8e4)

    # Dispatch: token list + gate weight per expert; overflow beyond C
    # falls back to an exact host computation (empty for the spec'd data).
    idx_list, w_list, overflow = [], [], []
    for e in range(E):
        toks, kpos = np.nonzero(topk_idx == e)
        we = topk_w[toks, kpos]
        if len(toks) > C:
            overflow.append((e, toks[C:], we[C:]))
            toks, we = toks[:C], we[:C]
        idx_list.append(toks)
        w_list.append(we)

    in_maps = []
    for e in range(E):
        toks = idx_list[e]
        n = len(toks)
        xg = np.zeros((C, H), dtype=f8e4)
        xg[:n] = x8[toks]
        xTd = np.zeros((P, HO, 288), dtype=f8e4)  # rows padded for DoubleRow
        xTd[:, :, :C] = xg.T.reshape(HO, P, C).transpose(1, 0, 2)
        wgd = np.ascontiguousarray(
            (w_gate[e].astype(np.float64) * W_SC).astype(f8e4)
            .reshape(HO, P, IO, P).transpose(1, 2, 0, 3))
        wud = np.ascontiguousarray(
            (w_up[e].astype(np.float64) * W_SC).astype(f8e4)
            .reshape(HO, P, IO, P).transpose(1, 2, 0, 3))
        wdd = np.ascontiguousarray(
            (w_down[e].astype(np.float64) * W_SC).astype(f8)
            .reshape(IO, P, HO, P).transpose(1, 2, 0, 3))
        in_maps.append({"xT": xTd, "wg": wgd, "wu": wud, "wd": wdd})

    if _PROGRAM is None:
        _PROGRAM = _build_program()

    from concourse.bass_utils import run_bass_kernel_spmd

    kw = {}
    if os.environ.get("MOE_KERNEL_TRACE", "") == "1":
        kw = dict(trace=True, trace_cores=list(range(N_CORES)))
    res = run_bass_kernel_spmd(
        _PROGRAM, in_maps, core_ids=list(range(N_CORES)), **kw)
    LAST_RESULTS = res

    out = np.zeros((T, H), dtype=np.float64)
    for e in range(E):
        n = len(idx_list[e])
        if n:
            ye = res.results[e]["y"]  # [P, HO, C] bf16, scaled by W_SC
            yf = ye.transpose(1, 0, 2).reshape(H, C)
            out[idx_list[e]] += (w_list[e] / W_SC)[:, None] \
                * yf[:, :n].T.astype(np.float64)
    for e, toks, ws in overflow:
        xt = x[toks].astype(np.float64)
        g = xt @ w_gate[e].astype(np.float64)
        u = xt @ w_up[e].astype(np.float64)
        mid = (g / (1.0 + np.exp(-g))) * u
        out[toks] += ws[:, None] * (mid @ w_down[e].astype(np.float64))
    out += zero_w[:, None] * x.astype(np.float64)
    return out.astype(np.float32)


# revision 46
# speedup vs baseline: 1.0053x; 1.0053x over previous
"""LongcatMoe Trainium2 kernel — expert-parallel sparse MoE across 8 NeuronCores.

Strategy (expert-parallel, per the sharding hint):
  - Host computes the tiny router (fp64 softmax/top-k, ~34 MFLOP) and
    dispatches tokens by top-k expert id: core e receives the tokens routed
    to expert e (padded to capacity C), plus expert e's weights.
  - Each core runs the silu-gated MLP for its expert on its token block in
    fp8 e3m4 (4-bit mantissa) with fp32 PSUM accumulation:
      y[:, t] = ((silu(Wg.T x_t)) * (Wu.T x_t)).T @ Wd     in [H, C] layout.
    e3m4 matmuls run at full bf16 PE rate (FWL weight loads) but halve HBM
    traffic, which removes the DMA bottleneck of the bf16 version.
  - Scaling: x*2, W*128 on host (exact powers of two); the PSUM descale
    (1/256) folds into the scalar-engine activations and the residual 128
    folds into the host-side combine weights, so no extra device ops.
  - Host combines in fp64: out[tok] += (gate_w/128) * y, plus the
    zero-expert (identity) term zero_w[t] * x[t] computed exactly on host.

All device tensors are partition-major so every DMA moves >=2KB-ish
contiguous runs per partition, and each logical tensor is ONE SBUF tile
loaded by a few chunked DMAs (fewer ring-paced issue instructions, and a
short tile-pool epilogue: the TileContext teardown semaphore chain scales
with the slot count).

  xT  [128, HO, C]        xT[p, h, t]   = e3m4(2 * x[idx[t], h*128+p])
  wg  [128, IO, HO, 128]  wg[p, j, h, c] = e3m4(128 * w_gate[h*128+p, j*128+c])
  wu  same as wg
  wd  [128, HO, IO, 128]  wd[p, k, j, c] = e3m4(128 * w_down[j*128+p, k*128+c])
  y   [128, HO, C] bf16   y[p, k, t]    = 128 * down[idx[t], k*128+p]
"""

import os

import numpy as np
import ml_dtypes

T, H, I, E, Z, TOPK = 1024, 2048, 1024, 8, 8, 4
ROUTED_SCALING = 1.0
N_CORES = 8
P = 128
HO = H // P  # 16
IO = I // P  # 8
C = 280      # per-expert token capacity on device (seed-0 max is 278)
X_SC = 2.0   # host scale on x before e3m4 quantization
W_SC = 128.0  # host scale on all weights before e3m4 quantization

_PROGRAM = None
LAST_RESULTS = None  # BassKernelResults of the most recent run (for test harness)


def _build_program():
    import concourse.mybir as mybir
    import concourse.tile as tile
    from concourse import bacc

    f32 = mybir.dt.float32
    bf16 = mybir.dt.bfloat16
    fp8 = mybir.dt.float8e3
    fp8e4 = mybir.dt.float8e4
    DR = mybir.MatmulPerfMode.DoubleRow
    SILU = mybir.ActivationFunctionType.Silu
    COPY = mybir.ActivationFunctionType.Copy
    DESC = 1.0 / (X_SC * W_SC)  # PSUM descale for gate/up

    nc = bacc.Bacc(
        "TRN2",
        target_bir_lowering=False,
        debug=False,
        enable_asserts=False,
        num_devices=N_CORES,
    )
    # x rows padded 280->288 so the DoubleRow moving AP's h-pair stride is
    # 16-aligned (DR constraint).
    CP = 288
    xT = nc.dram_tensor("xT", [P, HO, CP], fp8e4, kind="ExternalInput").ap()
    wg = nc.dram_tensor("wg", [P, IO, HO, P], fp8e4, kind="ExternalInput").ap()
    wu = nc.dram_tensor("wu", [P, IO, HO, P], fp8e4, kind="ExternalInput").ap()
    wd = nc.dram_tensor("wd", [P, HO, IO, P], fp8, kind="ExternalInput").ap()
    y = nc.dram_tensor("y", [P, HO, C], bf16, kind="ExternalOutput").ap()

    with tile.TileContext(nc) as tc:
        with (
            tc.tile_pool(name="px", bufs=1) as px,
            tc.tile_pool(name="pwg", bufs=1) as pwg,
            tc.tile_pool(name="pwu", bufs=1) as pwu,
            tc.tile_pool(name="pwd", bufs=1) as pwd,
            tc.tile_pool(name="pmid", bufs=1) as pmid,
            tc.tile_pool(name="psg", bufs=2) as psg,
            tc.tile_pool(name="psu", bufs=2) as psu,
            tc.tile_pool(name="py", bufs=2) as py,
            tc.tile_pool(name="pwrm", bufs=1) as pwrm,
            tc.tile_pool(name="ppg", bufs=3, space="PSUM") as ppg,
            tc.tile_pool(name="ppu", bufs=3, space="PSUM") as ppu,
            tc.tile_pool(name="ppd", bufs=2, space="PSUM") as ppd,
        ):
            # Warmup source: memset on gpsimd (ready earliest in preamble).
            wtile = pwrm.tile([P, C], bf16)
            nc.gpsimd.memset(wtile[:], 0.0)

            xt = px.tile([P, HO, CP], fp8e4)
            wg_t = pwg.tile([P, IO, HO, P], fp8e4)
            wu_t = pwu.tile([P, IO, HO, P], fp8e4)
            wd_t = pwd.tile([P, HO, IO, P], fp8)
            midt = pmid.tile([P, IO, C], fp8)

            # sync (SP) ring: wg0, x chunks, wg1..7, later y out. Fine-grained
            # early chunks so the first gate matmuls start as soon as
            # possible; later weights ride behind at ring rate.
            # The two tensors gating the first matmul group (wg0 and x) ride
            # DIFFERENT rings so their transfers overlap: sync carries all of
            # wg, scalar carries x then wu.
            for j in range(IO):
                nc.sync.dma_start(wg_t[:, j], wg[:, j])
            nc.scalar.dma_start(xt[:, 0:8, :], xT[:, 0:8, :])
            nc.scalar.dma_start(xt[:, 8:, :], xT[:, 8:, :])
            # Tiny dummy activation pulls the scalar ACT_TABLE_LOAD into the
            # preamble instead of stalling the first real silu.
            dumy = pwrm.tile([P, 4], f32, tag="dum")
            nc.scalar.activation(dumy[:], wtile[:, :4], SILU)
            # All wu up front: the scalar ring carries only wu (2.1MB), so
            # the 8 issues clear the queue well before the first activation,
            # and the ring FIFO delivers wu_j at full rate for the faster
            # DoubleRow phase-1 pace.
            for j in range(IO):
                nc.scalar.dma_start(wu_t[:, j], wu[:, j])

            # PE warmup: keep the tensor engine busy while the first input
            # DMAs land so the HAM clock-gate promotes to 8/8 (2.4 GHz)
            # before (not during) the real matmuls.
            pwm = ppd.tile([P, C], f32, tag="pd")
            NWARM = 16
            for w in range(NWARM):
                nc.tensor.matmul(pwm[:], wtile[:, :P], wtile[:],
                                 start=(w == 0), stop=(w == NWARM - 1))

            # Phase 1: mid[j] = silu(x @ Wg_j) * (x @ Wu_j) in [I, C] layout,
            # e4m3 DoubleRow matmuls (each contracts two h-planes).
            # PSUM holds 256*gate and 256*up; the activations descale.
            for j in range(IO):
                pg = ppg.tile([P, C], f32)
                pu = ppu.tile([P, C], f32)
                for hh in range(HO // 2):
                    nc.tensor.matmul(
                        pg[:], wg_t[:, j, 2 * hh:2 * hh + 2, :],
                        xt[:, 2 * hh:2 * hh + 2, :C],
                        start=(hh == 0), stop=(hh == HO // 2 - 1),
                        perf_mode=DR,
                    )
                for hh in range(HO // 2):
                    nc.tensor.matmul(
                        pu[:], wu_t[:, j, 2 * hh:2 * hh + 2, :],
                        xt[:, 2 * hh:2 * hh + 2, :C],
                        start=(hh == 0), stop=(hh == HO // 2 - 1),
                        perf_mode=DR,
                    )
                sg = psg.tile([P, C], f32)
                nc.scalar.activation(sg[:], pg[:], SILU, scale=DESC)
                su = psu.tile([P, C], f32)
                nc.scalar.activation(su[:], pu[:], COPY, scale=DESC)
                # wd rides the sync ring (FIFO behind the wg transfers, so it
                # can never delay phase 1) — the scalar ring stays wu-only,
                # which is what paces the up_j matmuls.
                if 2 <= j <= 5:
                    cnk = j - 2
                    nc.sync.dma_start(wd_t[:, 4 * cnk:4 * cnk + 4],
                                      wd[:, 4 * cnk:4 * cnk + 4])
                nc.vector.tensor_mul(out=midt[:, j, :], in0=sg[:], in1=su[:])

            # Phase 2: y[k] = sum_j Wd[j, k].T @ mid[j] in [H, C] layout
            # (= 128 * true output; host divides the combine weights).
            # y leaves in 2-k chunks so the final transfer after the last
            # matmul is small.
            ty = None
            for k in range(HO):
                if k % 2 == 0:
                    ty = py.tile([P, 2, C], bf16)
                pd = ppd.tile([P, C], f32, tag="pd")
                for j in range(IO):
                    nc.tensor.matmul(
                        pd[:], wd_t[:, k, j, :], midt[:, j, :],
                        start=(j == 0), stop=(j == IO - 1),
                    )
                nc.vector.tensor_copy(out=ty[:, k % 2, :], in_=pd[:])
                if k % 2 == 1:
                    nc.sync.dma_start(y[:, k - 1:k + 1, :], ty[:])

    nc.compile()
    return nc


def _route(x, router_w, corr_bias):
    """fp64 router: returns (topk_idx [T,K], topk_w [T,K])."""
    xl = x.astype(np.float64)
    logits = xl @ router_w.astype(np.float64).T
    logits -= logits.max(axis=1, keepdims=True)
    p = np.exp(logits)
    p /= p.sum(axis=1, keepdims=True)
    sel = p + corr_bias.astype(np.float64)
    topk_idx = np.argsort(-sel, axis=1, kind="stable")[:, :TOPK]
    topk_w = np.take_along_axis(p, topk_idx, axis=1) * ROUTED_SCALING
    return topk_idx, topk_w


def kernel(hidden_states, router_w, corr_bias, w_gate, w_up, w_down):
    global _PROGRAM, LAST_RESULTS
    x = np.asarray(hidden_states, dtype=np.float32)
    router_w = np.asarray(router_w, dtype=np.float32)
    corr_bias = np.asarray(corr_bias, dtype=np.float32)
    w_gate = np.asarray(w_gate, dtype=np.float32)
    w_up = np.asarray(w_up, dtype=np.float32)
    w_down = np.asarray(w_down, dtype=np.float32)

    topk_idx, topk_w = _route(x, router_w, corr_bias)
    routed = topk_idx < E
    zero_w = (topk_w * (~routed)).sum(axis=1)  # [T] fp64

    f8 = ml_dtypes.float8_e3m4
    f8e4 = ml_dtypes.float8_e4m3
    x8 = (x.astype(np.float64) * X_SC).astype(f8e4)

    # Dispatch: token list + gate weight per expert; overflow beyond C
    # falls back to an exact host computation (empty for the spec'd data).
    idx_list, w_list, overflow = [], [], []
    for e in range(E):
        toks, kpos = np.nonzero(topk_idx == e)
        we = topk_w[toks, kpos]
        if len(toks) > C:
            overflow.append((e, toks[C:], we[C:]))
            toks, we = toks[:C], we[:C]
        idx_list.append(toks)
        w_list.append(we)

    in_maps = []
    for e in range(E):
        toks = idx_list[e]
        n = len(toks)
        xg = np.zeros((C, H), dtype=f8e4)
        xg[:n] = x8[toks]
        xTd = np.zeros((P, HO, 288), dtype=f8e4)  # rows padded for DoubleRow
        xTd[:, :, :C] = xg.T.reshape(HO, P, C).transpose(1, 0, 2)
        wgd = np.ascontiguousarray(
            (w_gate[e].astype(np.float64) * W_SC).astype(f8e4)
            .reshape(HO, P, IO, P).transpose(1, 2, 0, 3))
        wud = np.ascontiguousarray(
            (w_up[e].astype(np.float64) * W_SC).astype(f8e4)
            .reshape(HO, P, IO, P).transpose(1, 2, 0, 3))
        wdd = np.ascontiguousarray(
            (w_down[e].astype(np.float64) * W_SC).astype(f8)
            .reshape(IO, P, HO, P).transpose(1, 2, 0, 3))
        in_maps.append({"xT": xTd, "wg": wgd, "wu": wud, "wd": wdd})

    if _PROGRAM is None:
        _PROGRAM = _build_program()

    from concourse.bass_utils import run_bass_kernel_spmd

    kw = {}
    if os.environ.get("MOE_KERNEL_TRACE", "") == "1":
        kw = dict(trace=True, trace_cores=list(range(N_CORES)))
    res = run_bass_kernel_spmd(
        _PROGRAM, in_maps, core_ids=list(range(N_CORES)), **kw)
    LAST_RESULTS = res

    out = np.zeros((T, H), dtype=np.float64)
    for e in range(E):
        n = len(idx_list[e])
        if n:
            ye = res.results[e]["y"]  # [P, HO, C] bf16, scaled by W_SC
            yf = ye.transpose(1, 0, 2).reshape(H, C)
            out[idx_list[e]] += (w_list[e] / W_SC)[:, None] \
                * yf[:, :n].T.astype(np.float64)
    for e, toks, ws in overflow:
        xt = x[toks].astype(np.float64)
        g = xt @ w_gate[e].astype(np.float64)
        u = xt @ w_up[e].astype(np.float64)
        mid = (g / (1.0 + np.exp(-g))) * u
        out[toks] += ws[:, None] * (mid @ w_down[e].astype(np.float64))
    out += zero_w[:, None] * x.astype(np.float64)
    return out.astype(np.float32)


# revision 48
# speedup vs baseline: 1.0751x; 1.0694x over previous
"""LongcatMoe Trainium2 kernel — expert-parallel sparse MoE across 8 NeuronCores.

Strategy (expert-parallel, per the sharding hint):
  - Host computes the tiny router (fp64 softmax/top-k, ~34 MFLOP) and
    dispatches tokens by top-k expert id: core e receives the tokens routed
    to expert e (padded to capacity C), plus expert e's weights.
  - Each core runs the silu-gated MLP for its expert on its token block in
    fp8 e3m4 (4-bit mantissa) with fp32 PSUM accumulation:
      y[:, t] = ((silu(Wg.T x_t)) * (Wu.T x_t)).T @ Wd     in [H, C] layout.
    e3m4 matmuls run at full bf16 PE rate (FWL weight loads) but halve HBM
    traffic, which removes the DMA bottleneck of the bf16 version.
  - Scaling: x*2, W*128 on host (exact powers of two); the PSUM descale
    (1/256) folds into the scalar-engine activations and the residual 128
    folds into the host-side combine weights, so no extra device ops.
  - Host combines in fp64: out[tok] += (gate_w/128) * y, plus the
    zero-expert (identity) term zero_w[t] * x[t] computed exactly on host.

All device tensors are partition-major so every DMA moves >=2KB-ish
contiguous runs per partition, and each logical tensor is ONE SBUF tile
loaded by a few chunked DMAs (fewer ring-paced issue instructions, and a
short tile-pool epilogue: the TileContext teardown semaphore chain scales
with the slot count).

  xT  [128, HO, C]        xT[p, h, t]   = e3m4(2 * x[idx[t], h*128+p])
  wg  [128, IO, HO, 128]  wg[p, j, h, c] = e3m4(128 * w_gate[h*128+p, j*128+c])
  wu  same as wg
  wd  [128, HO, IO, 128]  wd[p, k, j, c] = e3m4(128 * w_down[j*128+p, k*128+c])
  y   [128, HO, C] bf16   y[p, k, t]    = 128 * down[idx[t], k*128+p]
"""

import os

import numpy as np
import ml_dtypes

T, H, I, E, Z, TOPK = 1024, 2048, 1024, 8, 8, 4
ROUTED_SCALING = 1.0
N_CORES = 8
P = 128
HO = H // P  # 16
IO = I // P  # 8
C = 280      # per-expert token capacity on device (seed-0 max is 278)
X_SC = 2.0   # host scale on x before e3m4 quantization
W_SC = 128.0  # host scale on all weights before e3m4 quantization

_PROGRAM = None
LAST_RESULTS = None  # BassKernelResults of the most recent run (for test harness)


def _build_program():
    import concourse.mybir as mybir
    import concourse.tile as tile
    from concourse import bacc

    f32 = mybir.dt.float32
    bf16 = mybir.dt.bfloat16
    fp8 = mybir.dt.float8e3
    fp8e4 = mybir.dt.float8e4
    DR = mybir.MatmulPerfMode.DoubleRow
    SILU = mybir.ActivationFunctionType.Silu
    COPY = mybir.ActivationFunctionType.Copy
    DESC = 1.0 / (X_SC * W_SC)  # PSUM descale for gate/up

    nc = bacc.Bacc(
        "TRN2",
        target_bir_lowering=False,
        debug=False,
        enable_asserts=False,
        num_devices=N_CORES,
    )
    # x rows padded 280->288 so the DoubleRow moving AP's h-pair stride is
    # 16-aligned (DR constraint).
    CP = 288
    xT = nc.dram_tensor("xT", [P, HO, CP], fp8e4, kind="ExternalInput").ap()
    wg = nc.dram_tensor("wg", [P, IO, HO, P], fp8e4, kind="ExternalInput").ap()
    wu = nc.dram_tensor("wu", [P, IO, HO, P], fp8e4, kind="ExternalInput").ap()
    wd = nc.dram_tensor("wd", [P, HO, IO, P], fp8, kind="ExternalInput").ap()
    y = nc.dram_tensor("y", [P, HO, C], bf16, kind="ExternalOutput").ap()

    with tile.TileContext(nc) as tc:
        with (
            tc.tile_pool(name="px", bufs=1) as px,
            tc.tile_pool(name="pwg", bufs=1) as pwg,
            tc.tile_pool(name="pwu", bufs=1) as pwu,
            tc.tile_pool(name="pwd", bufs=1) as pwd,
            tc.tile_pool(name="pmid", bufs=1) as pmid,
            tc.tile_pool(name="psg", bufs=2) as psg,
            tc.tile_pool(name="psu", bufs=2) as psu,
            tc.tile_pool(name="py", bufs=2) as py,
            tc.tile_pool(name="pwrm", bufs=1) as pwrm,
            tc.tile_pool(name="ppg", bufs=3, space="PSUM") as ppg,
            tc.tile_pool(name="ppu", bufs=2, space="PSUM") as ppu,
            tc.tile_pool(name="ppd", bufs=3, space="PSUM") as ppd,
        ):
            # Warmup source: memset on gpsimd (ready earliest in preamble).
            wtile = pwrm.tile([P, C], bf16)
            nc.gpsimd.memset(wtile[:], 0.0)

            xt = px.tile([P, HO, CP], fp8e4)
            wg_t = pwg.tile([P, IO, HO, P], fp8e4)
            wu_t = pwu.tile([P, IO, HO, P], fp8e4)
            wd_t = pwd.tile([P, HO, IO, P], fp8)
            midt = pmid.tile([P, IO, C], fp8)

            # sync (SP) ring: wg0, x chunks, wg1..7, later y out. Fine-grained
            # early chunks so the first gate matmuls start as soon as
            # possible; later weights ride behind at ring rate.
            # The two tensors gating the first matmul group (wg0 and x) ride
            # DIFFERENT rings so their transfers overlap, and x's two halves
            # are split across rings so both carry the same early byte load
            # (2.39MB each): sync = wg0, x-tail, wg1..7; scalar = x-head, wu.
            nc.sync.dma_start(wg_t[:, 0], wg[:, 0])
            nc.sync.dma_start(xt[:, 8:, :], xT[:, 8:, :])
            for j in range(1, IO):
                nc.sync.dma_start(wg_t[:, j], wg[:, j])
            nc.scalar.dma_start(xt[:, 0:8, :], xT[:, 0:8, :])
            # Tiny dummy activation pulls the scalar ACT_TABLE_LOAD into the
            # preamble instead of stalling the first real silu.
            dumy = pwrm.tile([P, 4], f32, tag="dum")
            nc.scalar.activation(dumy[:], wtile[:, :4], SILU)
            # All wu up front: the scalar ring carries only wu (2.1MB), so
            # the 8 issues clear the queue well before the first activation,
            # and the ring FIFO delivers wu_j at full rate for the faster
            # DoubleRow phase-1 pace.
            for j in range(IO):
                nc.scalar.dma_start(wu_t[:, j], wu[:, j])

            # PE warmup: keep the tensor engine busy while the first input
            # DMAs land so the HAM clock-gate promotes to 8/8 (2.4 GHz)
            # before (not during) the real matmuls.
            pwm = ppd.tile([P, C], f32, tag="pd")
            NWARM = 16
            for w in range(NWARM):
                nc.tensor.matmul(pwm[:], wtile[:, :P], wtile[:],
                                 start=(w == 0), stop=(w == NWARM - 1))

            # Phase 1: mid[j] = silu(x @ Wg_j) * (x @ Wu_j) in [I, C] layout,
            # e4m3 DoubleRow matmuls (each contracts two h-planes).
            # PSUM holds 256*gate and 256*up; the activations descale.
            for j in range(IO):
                pg = ppg.tile([P, C], f32)
                pu = ppu.tile([P, C], f32)
                for hh in range(HO // 2):
                    nc.tensor.matmul(
                        pg[:], wg_t[:, j, 2 * hh:2 * hh + 2, :],
                        xt[:, 2 * hh:2 * hh + 2, :C],
                        start=(hh == 0), stop=(hh == HO // 2 - 1),
                        perf_mode=DR,
                    )
                for hh in range(HO // 2):
                    nc.tensor.matmul(
                        pu[:], wu_t[:, j, 2 * hh:2 * hh + 2, :],
                        xt[:, 2 * hh:2 * hh + 2, :C],
                        start=(hh == 0), stop=(hh == HO // 2 - 1),
                        perf_mode=DR,
                    )
                sg = psg.tile([P, C], f32)
                nc.scalar.activation(sg[:], pg[:], SILU, scale=DESC)
                su = psu.tile([P, C], f32)
                nc.scalar.activation(su[:], pu[:], COPY, scale=DESC)
                # wd rides the sync ring (FIFO behind the wg transfers, so it
                # can never delay phase 1) — the scalar ring stays wu-only,
                # which is what paces the up_j matmuls.
                if 2 <= j <= 5:
                    cnk = j - 2
                    nc.sync.dma_start(wd_t[:, 4 * cnk:4 * cnk + 4],
                                      wd[:, 4 * cnk:4 * cnk + 4])
                nc.vector.tensor_mul(out=midt[:, j, :], in0=sg[:], in1=su[:])

            # Phase 2: y[k] = sum_j Wd[j, k].T @ mid[j] in [H, C] layout
            # (= 128 * true output; host divides the combine weights).
            # y leaves in 2-k chunks so the final transfer after the last
            # matmul is small.
            ty = None
            for k in range(HO):
                if k % 2 == 0:
                    ty = py.tile([P, 2, C], bf16)
                pd = ppd.tile([P, C], f32, tag="pd")
                for j in range(IO):
                    nc.tensor.matmul(
                        pd[:], wd_t[:, k, j, :], midt[:, j, :],
                        start=(j == 0), stop=(j == IO - 1),
                    )
                nc.vector.tensor_copy(out=ty[:, k % 2, :], in_=pd[:])
                if k % 2 == 1:
                    nc.sync.dma_start(y[:, k - 1:k + 1, :], ty[:])

    nc.compile()
    return nc


def _route(x, router_w, corr_bias):
    """fp64 router: returns (topk_idx [T,K], topk_w [T,K])."""
    xl = x.astype(np.float64)
    logits = xl @ router_w.astype(np.float64).T
    logits -= logits.max(axis=1, keepdims=True)
    p = np.exp(logits)
    p /= p.sum(axis=1, keepdims=True)
    sel = p + corr_bias.astype(np.float64)
    topk_idx = np.argsort(-sel, axis=1, kind="stable")[:, :TOPK]
    topk_w = np.take_along_axis(p, topk_idx, axis=1) * ROUTED_SCALING
    return topk_idx, topk_w


def kernel(hidden_states, router_w, corr_bias, w_gate, w_up, w_down):
    global _PROGRAM, LAST_RESULTS
    x = np.asarray(hidden_states, dtype=np.float32)
    router_w = np.asarray(router_w, dtype=np.float32)
    corr_bias = np.asarray(corr_bias, dtype=np.float32)
    w_gate = np.asarray(w_gate, dtype=np.float32)
    w_up = np.asarray(w_up, dtype=np.float32)
    w_down = np.asarray(w_down, dtype=np.float32)

    topk_idx, topk_w = _route(x, router_w, corr_bias)
    routed = topk_idx < E
    zero_w = (topk_w * (~routed)).sum(axis=1)  # [T] fp64

    f8 = ml_dtypes.float8_e3m4
    f8e4 = ml_dtypes.float8_e4m3
    x8 = (x.astype(np.float64) * X_SC).astype(f8e4)

    # Dispatch: token list + gate weight per expert; overflow beyond C
    # falls back to an exact host computation (empty for the spec'd data).
    idx_list, w_list, overflow = [], [], []
    for e in range(E):
        toks, kpos = np.nonzero(topk_idx == e)
        we = topk_w[toks, kpos]
        if len(toks) > C:
            overflow.append((e, toks[C:], we[C:]))
            toks, we = toks[:C], we[:C]
        idx_list.append(toks)
        w_list.append(we)

    in_maps = []
    for e in range(E):
        toks = idx_list[e]
        n = len(toks)
        xg = np.zeros((C, H), dtype=f8e4)
        xg[:n] = x8[toks]
        xTd = np.zeros((P, HO, 288), dtype=f8e4)  # rows padded for DoubleRow
        xTd[:, :, :C] = xg.T.reshape(HO, P, C).transpose(1, 0, 2)
        wgd = np.ascontiguousarray(
            (w_gate[e].astype(np.float64) * W_SC).astype(f8e4)
            .reshape(HO, P, IO, P).transpose(1, 2, 0, 3))
        wud = np.ascontiguousarray(
            (w_up[e].astype(np.float64) * W_SC).astype(f8e4)
            .reshape(HO, P, IO, P).transpose(1, 2, 0, 3))
        wdd = np.ascontiguousarray(
            (w_down[e].astype(np.float64) * W_SC).astype(f8)
            .reshape(IO, P, HO, P).transpose(1, 2, 0, 3))
        in_maps.append({"xT": xTd, "wg": wgd, "wu": wud, "wd": wdd})

    if _PROGRAM is None:
        _PROGRAM = _build_program()

    from concourse.bass_utils import run_bass_kernel_spmd

    kw = {}
    if os.environ.get("MOE_KERNEL_TRACE", "") == "1":
        kw = dict(trace=True, trace_cores=list(range(N_CORES)))
    res = run_bass_kernel_spmd(
        _PROGRAM, in_maps, core_ids=list(range(N_CORES)), **kw)
    LAST_RESULTS = res

    out = np.zeros((T, H), dtype=np.float64)
    for e in range(E):
        n = len(idx_list[e])
        if n:
            ye = res.results[e]["y"]  # [P, HO, C] bf16, scaled by W_SC
            yf = ye.transpose(1, 0, 2).reshape(H, C)
            out[idx_list[e]] += (w_list[e] / W_SC)[:, None] \
                * yf[:, :n].T.astype(np.float64)
    for e, toks, ws in overflow:
        xt = x[toks].astype(np.float64)
        g = xt @ w_gate[e].astype(np.float64)
        u = xt @ w_up[e].astype(np.float64)
        mid = (g / (1.0 + np.exp(-g))) * u
        out[toks] += ws[:, None] * (mid @ w_down[e].astype(np.float64))
    out += zero_w[:, None] * x.astype(np.float64)
    return out.astype(np.float32)
